# revision 13
# baseline (speedup 1.0000x reference)
"""Rank-1 triangular attention via Taylor moments, v3.

Math (per batch): k = x @ wk, q = x @ wq, c = q/32; column-softmax over
i <= j of exp(k_i c_j); out[j] = sum_i a[i,j] f[i].  |k c| <= ~0.21 so
exp(k_i c_j) = sum_p (k_i^p/p!) c_j^p (5 terms, err ~3e-6).  Off-diagonal
block contributions become moment prefix sums C_p[d] = sum_i k_i^p f[i,d];
the diagonal 128x128 block uses exact exp with a triangular mask.

v3 changes vs v2 (v2 ran ~100-134us, PE HAM-throttled to 1.2 GHz with
~2us/tile PE stalls and ~2.9us/tile of small DVE/ACT copies):
  - moments accumulate as TWO 512-col halves plus an H strip in one psum
    bank (PE col strips 0-4 / 32-36 / 64-68 run concurrently); snapshots
    shrink from 5 copies to 2x[5,512]+[5,1] into a flat c_sb [5,1025].
  - V^T C becomes two K=5 matmuls (lhsT = cpT [5,128]) against the flat
    c_sb -- the 101-row zero-padded v4, its big copy, and the 4 col-group
    transposes are gone (one [5,128] transpose instead).
  - stage A k/q uses fp8 DoubleRow matmuls (K=256/instr, 2 cols/cycle)
    with M=2 output partitions, fed by a host-packed pair layout.
  - the loop is software-pipelined one tile ahead: cb/exp/mask/cpT for
    tile t+1 are emitted while tile t's big matmuls run, so the PE never
    waits on ACT/GpSimd and stays HAM-warm at 2.4 GHz.
"""

import sys

sys.path.insert(0, "/opt/trn_rl_repo")

import numpy as np
import ml_dtypes

B, N, D = 8, 2048, 1024
P = 128
NT = N // P       # 16 i/j tiles
PD = 5            # Taylor terms p=0..4
HALF = D // 2     # 512
NB = N // 256     # 8 stage-A n-blocks

_CACHE = {}


def _patch_compiler_flags():
    from concourse import compiler_utils as cu

    flags = [f.replace("--enable-ldw-opt=false", "--enable-ldw-opt=true")
             for f in cu.get_compiler_flags()]
    cu.set_compiler_flags(flags)


def _build():
    import concourse.bacc as bacc
    import concourse.mybir as mybir
    from concourse.tile import TileContext
    from concourse.masks import make_identity, make_upper_triangular

    _patch_compiler_flags()

    dt = mybir.dt
    f32 = dt.float32
    bf16 = dt.bfloat16
    fp8 = dt.float8e4
    AF = mybir.ActivationFunctionType
    ALU = mybir.AluOpType
    DR = mybir.MatmulPerfMode.DoubleRow

    nc = bacc.Bacc(None, target_bir_lowering=False)
    # xt: [p, (c,i,n)] with d = (2c+i)*128+p, host-packed for DoubleRow
    xt_ext = nc.declare_dram_parameter("xt", [P, 4 * 4 * 2 * 512], fp8,
                                       isOutput=False)
    f_ext = nc.declare_dram_parameter("f", [N, D], bf16, isOutput=False)
    # w: [p, (c,i,m)] m in {32k, 32q}
    w_ext = nc.declare_dram_parameter("w", [P, 4 * 2 * 16], fp8,
                                      isOutput=False)
    out_ext = nc.declare_dram_parameter("out", [N, D], bf16, isOutput=True)

    with TileContext(nc) as tc:
        with (
            tc.tile_pool(name="const", bufs=1) as cpool,
            tc.tile_pool(name="fin", bufs=3) as fpool,
            tc.tile_pool(name="sd", bufs=3) as sdpool,
            tc.tile_pool(name="outsb", bufs=3) as opool,
            tc.tile_pool(name="csb", bufs=3) as cspool,
            tc.tile_pool(name="v4p", bufs=3) as v4pool,
            tc.tile_pool(name="rz", bufs=3) as rzpool,
            tc.tile_pool(name="ps_C", bufs=1, space="PSUM") as ps_C_pool,
            tc.tile_pool(name="ps_out", bufs=4, space="PSUM") as ps_out_pool,
            tc.tile_pool(name="ps_vt", bufs=1, space="PSUM") as ps_vt_pool,
            tc.tile_pool(name="ps_small", bufs=2, space="PSUM") as ps_small_pool,
        ):
            # ---------------- constants ----------------
            ident_bf = cpool.tile([P, P], bf16, tag="ident_bf")
            make_identity(nc, ident_bf[:])
            triu = cpool.tile([P, P], bf16, tag="triu")
            make_upper_triangular(nc, triu[:], val=1.0, diag=True)
            # 64 columns of ones: tiny matmuls are widened to 64 cols so
            # the PE array stays busy (HAM activity) at negligible cost
            ones_col = cpool.tile([P, 64], bf16, tag="ones_col")
            nc.gpsimd.memset(ones_col[:], 1.0)

            # persistent moment accumulator, one bank:
            # rows 0-4  cols 0:512 = C_p for d in [0,512)
            # rows 32-36 cols 0:512 = C_p for d in [512,1024)
            # rows 64-68 col 0      = H_p = sum_i k_i^p
            psum_C = ps_C_pool.tile([P, HALF], f32, tag="psC",
                                    name="psC")

            # PE warm-up: garbage matmuls release the HAM clock-gate while
            # the xt DMA streams in.
            warm_ps = ps_out_pool.tile([P, HALF], f32, tag="out_ps",
                                       name="warm")
            for _ in range(26):
                nc.tensor.matmul(
                    warm_ps[:, 0:P], lhsT=ident_bf[:], rhs=triu[:],
                    start=True, stop=True, skip_group_check=True,
                )

            # ---------------- stage A: k,q ----------------
            w_sb = cpool.tile([P, 4, 2, 16], fp8, tag="w_sb")
            nc.sync.dma_start(
                out=w_sb[:], in_=w_ext.rearrange("p (c i m) -> p c i m",
                                                 c=4, i=2))
            # xt_sb[p, j, c, i, n512]: n = 512*j + n512, d = (2c+i)*128+p
            xt_sb = cpool.tile([P, 4, 4, 2, 512], fp8, tag="xt_sb")
            CH = 4 * 2 * 512
            for j in range(4):
                nc.sync.dma_start(
                    out=xt_sb[:, j, :, :, :],
                    in_=xt_ext[:, j * CH:(j + 1) * CH].rearrange(
                        "p (c i n) -> p c i n", c=4, i=2),
                )

            # kq_sb rows: 0 = 32q, 1 = 32k (host pre-scales w by 32)
            # cb_all[i, n] = c_n = q_n/32: K=1 matmul broadcast of kq row 0
            # (scaled by the 1/1024-valued stationary row), built per 512-col
            # chunk as soon as its two kq blocks land
            kq_sb = cpool.tile([2, N], bf16, tag="kq_sb")
            ones_row = cpool.tile([1, P], bf16, tag="ones_row")
            nc.gpsimd.memset(ones_row[:], 1.0 / 1024.0)
            cb_all = cpool.tile([P, N], bf16, tag="cb_all")
            for nb in range(NB):
                kq_ps = ps_small_pool.tile([16, 256], f32, tag="smalls",
                                           name=f"kq_ps{nb}")
                j, o = nb // 2, (nb % 2) * 256
                for c in range(4):
                    nc.tensor.matmul(
                        kq_ps[:], lhsT=w_sb[:, c, :, :],
                        rhs=xt_sb[:, j, c, :, o:o + 256],
                        start=(c == 0), stop=(c == 3), perf_mode=DR,
                    )
                dst = kq_sb[:, nb * 256:(nb + 1) * 256]
                if nb % 2 == 0:
                    nc.vector.tensor_copy(dst, kq_ps[0:2, :])
                else:
                    nc.scalar.copy(dst, kq_ps[0:2, :])
                    s = nb // 2
                    cb_ps = ps_out_pool.tile([P, HALF], f32, tag="out_ps",
                                             name=f"cb_ps{s}")
                    nc.tensor.matmul(
                        cb_ps[:], lhsT=ones_row[:],
                        rhs=kq_sb[0:1, s * HALF:(s + 1) * HALF],
                        start=True, stop=True, skip_group_check=True,
                    )
                    if s % 2 == 0:
                        nc.vector.tensor_copy(
                            cb_all[:, s * HALF:(s + 1) * HALF], cb_ps[:])
                    else:
                        nc.scalar.copy(
                            cb_all[:, s * HALF:(s + 1) * HALF], cb_ps[:])

            for _ in range(10):
                nc.tensor.matmul(
                    warm_ps[:, 0:P], lhsT=ident_bf[:], rhs=triu[:],
                    start=True, stop=True, skip_group_check=True,
                )

            # per-tile k/c columns: PE transpose of [2,128] slices, batched
            # into one psum tile then one copy
            kc_ps = ps_small_pool.tile([P, 2 * NT], f32, tag="smalls",
                                       name="kc_ps")
            for t in range(NT):
                nc.tensor.matmul(
                    kc_ps[:, 2 * t:2 * t + 2],
                    lhsT=kq_sb[:, t * P:(t + 1) * P],
                    rhs=ident_bf[0:2, 0:2], start=True, stop=True,
                    skip_group_check=True,
                )
            kc_all = cpool.tile([P, NT, 2], f32, tag="kc_all")
            nc.vector.tensor_copy(
                kc_all[:], kc_ps[:].rearrange("p (t c) -> p t c", c=2))

            # bulk power tables over all tiles at once ([128,16] ops)
            kT = cpool.tile([P, NT], f32, tag="kT")   # true k (for exp scale)
            nc.vector.tensor_scalar_mul(kT[:], kc_all[:, :, 1], 1.0 / 32.0)
            kp_all = cpool.tile([P, NT, PD], bf16, tag="kp_all")
            cp_all = cpool.tile([P, NT, PD], bf16, tag="cp_all")
            nc.gpsimd.memset(kp_all[:, :, 0:1], 1.0)
            nc.gpsimd.memset(cp_all[:, :, 0:1], 1.0)
            nc.vector.tensor_copy(kp_all[:, :, 1], kT[:])
            nc.vector.tensor_scalar_mul(cp_all[:, :, 1], kc_all[:, :, 0],
                                        1.0 / 1024.0)
            # kp: k^p/p! ; cp: c^p (factorials folded into the k side)
            nc.vector.scalar_tensor_tensor(
                out=kp_all[:, :, 2], in0=kp_all[:, :, 1], scalar=0.5,
                in1=kp_all[:, :, 1], op0=ALU.mult, op1=ALU.mult)
            nc.vector.scalar_tensor_tensor(
                out=kp_all[:, :, 3], in0=kp_all[:, :, 2], scalar=1.0 / 3.0,
                in1=kp_all[:, :, 1], op0=ALU.mult, op1=ALU.mult)
            nc.vector.scalar_tensor_tensor(
                out=kp_all[:, :, 4], in0=kp_all[:, :, 3], scalar=0.25,
                in1=kp_all[:, :, 1], op0=ALU.mult, op1=ALU.mult)
            nc.vector.scalar_tensor_tensor(
                out=cp_all[:, :, 2], in0=cp_all[:, :, 1], scalar=1.0,
                in1=cp_all[:, :, 1], op0=ALU.mult, op1=ALU.mult)
            nc.vector.scalar_tensor_tensor(
                out=cp_all[:, :, 3], in0=cp_all[:, :, 2], scalar=1.0,
                in1=cp_all[:, :, 1], op0=ALU.mult, op1=ALU.mult)
            nc.vector.scalar_tensor_tensor(
                out=cp_all[:, :, 4], in0=cp_all[:, :, 3], scalar=1.0,
                in1=cp_all[:, :, 1], op0=ALU.mult, op1=ALU.mult)

            # bridge the DVE table-build phase with back-to-back spacer
            # matmuls into the spare psum_C partitions 96-127: a >1.4us PE
            # idle here trips the HAM MID window and the clock throttles for
            # the rest of the kernel
            def spacer(n):
                for sp_i in range(n):
                    nc.tensor.matmul(
                        psum_C[96:112, 0:256], lhsT=w_sb[:, 0, 0, :],
                        rhs=xt_sb[:, 0, 0, 0, 0:256], start=True, stop=True,
                        skip_group_check=True, tile_position=(0, 96),
                    )

            spacer(8)

            # ---------------- pipelined prologue for tile 0 ----------------
            def emit_sd(t):
                """exp -> triu mask for tile t; returns s_d tile."""
                s_d = sdpool.tile([P, P], bf16, tag="sd", name=f"sd{t}")
                nc.scalar.activation(s_d[:], cb_all[:, t * P:(t + 1) * P],
                                     AF.Exp, scale=kT[:, t:t + 1])
                nc.gpsimd.tensor_mul(s_d[:], s_d[:], triu[:])
                return s_d

            def emit_v4(t):
                """cpT transpose + copy for tile t; returns v4 [5,128]."""
                vt_ps = ps_vt_pool.tile([PD, P], f32, tag="vt", name=f"vt{t}")
                nc.tensor.matmul(
                    vt_ps[:], lhsT=cp_all[:, t, :], rhs=ident_bf[:],
                    start=True, stop=True, skip_group_check=True,
                )
                v4 = v4pool.tile([PD, P], bf16, tag="v4", name=f"v4{t}")
                nc.vector.tensor_copy(v4[:], vt_ps[:])
                return v4

            sd_q = [emit_sd(0), emit_sd(1)]
            v4_q = [emit_v4(0), emit_v4(1)]
            spacer(6)

            # ---------------- main loop ----------------
            fq = None
            o_sb = None
            cs_next = None
            for t in range(NT):
                if t % 4 == 0:
                    g = t // 4
                    fq = fpool.tile([P, 4, D], bf16, tag="fq", name=f"fq{g}")
                    # floor f loads so the xt load (which gates all compute)
                    # gets HBM bandwidth first
                    with tc.tile_wait_until(0.010 + 0.004 * g):
                        nc.sync.dma_start(
                            out=fq[:],
                            in_=f_ext[g * 4 * P:(g + 1) * 4 * P, :].rearrange(
                                "(u p) d -> p u d", p=P))
                    o_sb = opool.tile([P, 4, D], bf16, tag="o", name=f"o{g}")
                f_t = fq[:, t % 4, :]
                s_d = sd_q.pop(0)
                v4 = v4_q.pop(0)

                # c_sb for tile t was snapshotted at the end of iteration
                # t-1 (a full iteration of slack before VTC consumes it)
                c_sb = cs_next
                cs_next = None

                # pipeline tile t+2's ACT/GpSimd work behind tile t's matmuls
                if t + 2 < NT:
                    sd_q.append(emit_sd(t + 2))
                    v4_q.append(emit_v4(t + 2))

                # out = (s_d^T f_t + cpT^T C) / z
                u = t % 4
                out_pss = []
                for h in range(2):
                    out_ps = ps_out_pool.tile([P, HALF], f32, tag="out_ps",
                                              name=f"out_ps{t}_{h}")
                    nc.tensor.matmul(
                        out_ps[:], lhsT=s_d[:],
                        rhs=f_t[:, h * HALF:(h + 1) * HALF],
                        start=True, stop=(t == 0), skip_group_check=True,
                    )
                    if t >= 1:
                        nc.tensor.matmul(
                            out_ps[:], lhsT=v4[:],
                            rhs=c_sb[:, h * HALF:(h + 1) * HALF],
                            start=False, stop=True, skip_group_check=True,
                        )
                    out_pss.append(out_ps)
                # (halves emitted as diag->VTC pairs: VTC h0 runs while the
                # ACT snapshot of half 1 is still usable later)

                # z_j = sum_i s_d[i,j] + sum_p c_j^p H_p
                zc_ps = ps_small_pool.tile([P, 64], f32, tag="smalls",
                                           name=f"zc{t}")
                nc.tensor.matmul(
                    zc_ps[:], lhsT=s_d[:], rhs=ones_col[:], start=True,
                    stop=(t == 0), skip_group_check=True,
                )
                if t >= 1:
                    nc.tensor.matmul(
                        zc_ps[:, 0:1], lhsT=v4[:], rhs=c_sb[:, D:D + 1],
                        start=False, stop=True, skip_group_check=True,
                    )
                rz = rzpool.tile([P, 1], f32, tag="rz", name=f"rz{t}")
                nc.vector.reciprocal(rz[:], zc_ps[:, 0:1])

                # moment accumulation for tile t (t=15 is never consumed);
                # must come after the snapshot reads of prefix < t
                if t < NT - 1:
                    st, sp = (t == 0), (t == NT - 2)
                    for h in range(2):
                        nc.tensor.matmul(
                            psum_C[32 * h:32 * h + PD, :],
                            lhsT=kp_all[:, t, :],
                            rhs=f_t[:, h * HALF:(h + 1) * HALF],
                            start=st, stop=sp, skip_group_check=True,
                            tile_position=(0, 32 * h),
                        )
                    nc.tensor.matmul(
                        psum_C[64:64 + PD, 0:64], lhsT=kp_all[:, t, :],
                        rhs=ones_col[:], start=st, stop=sp,
                        skip_group_check=True, tile_position=(0, 64),
                    )

                nc.scalar.activation(o_sb[:, u, 0:HALF], out_pss[0][:],
                                     AF.Copy, scale=rz[:])
                nc.vector.tensor_scalar_mul(o_sb[:, u, HALF:D],
                                            out_pss[1][:], rz[:])

                # snapshot prefix (tiles <= t) for tile t+1:
                # c_sb = [C(d<512) 512 | C(d>=512) 512 | H 1]
                if t + 1 < NT:
                    cs_next = cspool.tile([PD, D + 1], bf16, tag="csb",
                                          name=f"csb{t + 1}")
                    nc.vector.tensor_copy(cs_next[:, D:D + 1],
                                          psum_C[64:64 + PD, 0:1])
                    nc.scalar.copy(cs_next[:, 0:HALF], psum_C[0:PD, :])
                    nc.vector.tensor_copy(cs_next[:, HALF:D],
                                          psum_C[32:32 + PD, :])

                if t % 4 == 3:
                    g = t // 4
                    nc.sync.dma_start(
                        out=out_ext[g * 4 * P:(g + 1) * 4 * P, :].rearrange(
                            "(u p) d -> p u d", p=P),
                        in_=o_sb[:])

    nc.compile()
    return nc


def _get_nc():
    if "nc" not in _CACHE:
        _CACHE["nc"] = _build()
    return _CACHE["nc"]


def kernel(x, f, wk, wq, trace=False):
    from concourse.bass_utils import run_bass_kernel_spmd

    x = np.asarray(x, dtype=np.float32)
    f = np.asarray(f, dtype=np.float32)
    wk = np.asarray(wk, dtype=np.float32)
    wq = np.asarray(wq, dtype=np.float32)

    bf = ml_dtypes.bfloat16
    f8 = ml_dtypes.float8_e4m3
    # xt pair layout: xt_p[b][p, c, i, n] = x[b, n, (2c+i)*128+p]
    xt = np.ascontiguousarray(np.transpose(x, (0, 2, 1)))  # [B, D, N]
    # -> [B, p, j, c, i, n512]
    xt_p = xt.reshape(B, 4, 2, P, 4, 512).transpose(0, 3, 4, 1, 2, 5)
    xt_p = np.ascontiguousarray(
        xt_p.reshape(B, P, 4 * 4 * 2 * 512)).astype(f8)
    fb = f.astype(bf)
    w = np.zeros((D, 16), dtype=np.float32)
    w[:, 0] = 32.0 * wq[0]
    w[:, 1] = 32.0 * wk[0]
    w_p = w.reshape(4, 2, P, 16).transpose(2, 0, 1, 3)
    w_p = np.ascontiguousarray(w_p.reshape(P, 4 * 2 * 16)).astype(f8)

    nc = _get_nc()
    in_maps = [{"xt": xt_p[b], "f": fb[b], "w": w_p} for b in range(B)]
    res = run_bass_kernel_spmd(nc, in_maps, core_ids=list(range(B)),
                               trace=trace)
    out = np.stack(
        [res.results[b]["out"].astype(np.float32) for b in range(B)], axis=0)
    if trace:
        _CACHE["last_exec_time_ns"] = res.exec_time_ns
        _CACHE["last_results"] = res
    return out
